# revision 56
# baseline (speedup 1.0000x reference)
"""Trainium2 Bass kernel for nn_MultiHeadAttention_7584912245188.

Reference computes (no softmax!):
    qkv = x @ Wqkv + bqkv ; split q,k,v ; per head: y = (q k^T / sqrt(D)) v
    out = y @ Wff + bff

Because there is no softmax, attention is linear and reassociates:
    (Q K^T) V = Q (K^T V).
With X_aug = [X | 1] ([N, 97]) and G = X_aug^T X_aug ([97, 97]), the whole
module collapses (associativity, per head h):
    out = X_aug @ Wfin,   Wfin = sum_h P_h G Q_h + e_last bff^T
    P_h = Wq_aug_h Wk_aug_h^T [97,97],  Q_h = D^-0.5 Wv_aug_h Wff_h [97,96]
P_h / Q_h are host-precomputed from the weights. On device per batch:
    G (16 accumulating matmuls over row chunks)
    R = G @ Qcat                     (3 matmuls, free dim 192 each)
    Wfin = sum_h P_h R_h + bff term  (7 PSUM-accumulating matmuls)
    out chunks = X_chunk @ Wfin      (via PE-transposed X chunks)
O(N*E^2) instead of O(N^2*D).

Sharding (8 cores): core c -> (batch b = c//2, sequence half h = c%2).
Each core receives x[b] (ones column appended host-side) rolled so "its"
half comes first, computes G from the full batch (redundantly within the
pair - cheaper than a collective), and writes only its half of the output.

This version is latency-optimized against the TimelineSim cost model
(every DMA pays descgen + 650ns DGE delay + payloads serialized on one
DMA_ENGINES device + 900ns completion-sem; every engine hop ~150ns):

- My half rides the first HWDGE slot; the other (Gram-only) half goes
  SWDGE so its descriptor generation overlaps the first HWDGE window and
  the two x payloads run back-to-back at full DMA bandwidth.
- Weights are split: the Q part (needed first, for R) rides the
  Activation engine's HWDGE slot right behind x; the P part (needed
  later, for Wfin) follows on SP. Neither gates the post-Gram chain,
  and the bias matmul is ordered LAST in the Wfin accumulation group so
  a late P-part cannot stall the P_h matmuls.
- PSUM->SBUF copies alternate between the two engines that can read
  PSUM (DVE and Activation) so the serial chain G -> R -> Wfin ->
  finals never waits on a busy copy engine; scheduler fences
  (tc.no_sync_barrier) pin the critical r0-copy ahead of the bulk XT
  transpose copies in the DVE queue.
- The framework's dead const-AP init memsets (no readers - walrus
  confirms) are stripped post-compile; they otherwise delay the entry
  barrier and every DMA by ~400ns.

A prepared-SWDGE scatter output path (trigger_dma) and fp8/DoubleRow
Gram variants were ~1.2us faster in the cost model but produced
compile-dependent numerical corruption on real hardware, so the output
uses the standard HWDGE DMA and all operands stay fp16 (f32 PSUM):
rel err 5.65e-4, deterministic across compiles.
"""

import numpy as np
from contextlib import ExitStack

import concourse.bass as bass
import concourse.tile as tile
from concourse import bacc, mybir
from concourse import bass_utils
from concourse.masks import make_identity

B, N, E = 4, 2048, 96
H = 6
D = E // H            # 16
P = 128
NCH = N // P          # 16 chunks of 128 rows
HALF = NCH // 2       # 8 chunks per core
EA = E + 1            # 97 (augmented with ones column)
SCALE = float(D) ** -0.5
F32 = mybir.dt.float32
F16 = mybir.dt.float16
I16 = mybir.dt.int16

# weight packing
WQ_COLS = H * E                      # 576  (Qcat)
C_OH = H * EA                        # 582  (PcatT | onehot | bff)
C_BF = C_OH + EA                     # 679
WP_COLS = C_BF + E                   # 775

N_CORES = 8

_NC_CACHE = {}
LAST_RESULTS = None


def _build_nc():
    nc = bacc.Bacc(
        "TRN2", target_bir_lowering=False, debug=False, num_devices=N_CORES
    )
    x = nc.dram_tensor("x", [N // 2, EA], F16, kind="ExternalInput").ap()
    xb = nc.dram_tensor("xb", [N // 2, EA], F16, kind="ExternalInput").ap()
    wq = nc.dram_tensor("wq", [EA, WQ_COLS], F16, kind="ExternalInput").ap()
    wp = nc.dram_tensor("wp", [EA, WP_COLS], F16, kind="ExternalInput").ap()
    # viewed [128, 768]: block-row p = out rows 8p..8p+7  (host reshapes)
    out = nc.dram_tensor("out", [P, HALF * E], F16, kind="ExternalOutput").ap()

    with tile.TileContext(nc) as tc, ExitStack() as ctx:
        sb = ctx.enter_context(tc.tile_pool(name="sb", bufs=1))
        ps_a = ctx.enter_context(tc.tile_pool(name="ps_a", bufs=4, space="PSUM"))
        ps_b = ctx.enter_context(tc.tile_pool(name="ps_b", bufs=3, space="PSUM"))
        ps_w = ctx.enter_context(tc.tile_pool(name="ps_w", bufs=1, space="PSUM"))

        # ---- input DMAs (per-engine emission order == queue order) ----
        # My half rides the first HWDGE slot (first payload); the other
        # half (Gram-only) goes SWDGE so its descgen overlaps; weights ride
        # the Act engine's HWDGE slot right behind them.
        xh = x.rearrange("(p j) e -> p j e", j=HALF)
        XA = sb.tile([P, HALF, EA], F16)
        nc.sync.dma_start(out=XA[:], in_=xh)                    # SP  HWDGE #1
        xbh = xb.rearrange("(p j) e -> p j e", j=HALF)
        NB1, NB2 = 5, 3
        XB1 = sb.tile([P, NB1, EA], F16)
        nc.gpsimd.dma_start(out=XB1[:], in_=xbh[:, 0:NB1, :])   # SWDGE
        # the small B tail rides the Act engine's first HWDGE slot (ready
        # in time to extend the payload train); with only 3 chunks after
        # its completion sem, the Gram tail shrinks by ~150ns
        XB2 = sb.tile([P, NB2, EA], F16)
        nc.scalar.dma_start(out=XB2[:], in_=xbh[:, NB1:HALF, :])  # Act HWDGE
        WQ = sb.tile([EA, WQ_COLS], F16)
        nc.sync.dma_start(out=WQ[:], in_=wq)                    # SP HWDGE #2
        WP = sb.tile([EA, WP_COLS], F16)
        nc.sync.dma_start(out=WP[:], in_=wp)                    # SP HWDGE #3

        # ---- Pool-engine setup work (all idle-time) ----
        id_sb = sb.tile([P, P], F16)
        make_identity(nc, id_sb[:])                             # gpsimd
        osb = sb.tile([P, HALF, E], F16)
        osb2d = osb[:].rearrange("p a b -> p (a b)")

        def Xc(c):
            if c < HALF:
                return XA[:, c, :]
            if c < HALF + NB1:
                return XB1[:, c - HALF, :]
            return XB2[:, c - HALF - NB1, :]

        # ---- G = X_aug^T X_aug: one 16-matmul fp8 PSUM accumulation ----
        g_ps = ps_b.tile([EA, EA], F32, tag="b", name="g_ps")
        for c in range(NCH):
            nc.tensor.matmul(
                g_ps[:], lhsT=Xc(c), rhs=Xc(c),
                start=(c == 0), stop=(c == NCH - 1),
            )

        # ---- g copy split across Act+DVE (halves finish ~40ns sooner) ----
        g_h = sb.tile([EA, EA], F16)
        nc.scalar.copy(out=g_h[:, 0:48], in_=g_ps[:, 0:48])
        nc.vector.tensor_copy(out=g_h[:, 48:EA], in_=g_ps[:, 48:EA])

        # ---- R = G @ Qcat in 2 pieces; copies fan out to DVE/Act ----
        # (GPSIMD/Pool cannot touch PSUM, so only two copy engines exist.)
        # R runs on PE as soon as g_h lands; the transposes (whose fp16
        # input arrives later anyway) follow behind a scheduler fence so
        # they cannot be hoisted in front of R.
        r_h = sb.tile([EA, H * E], F16)
        RW = H * E // 2                                         # 288
        r0 = ps_b.tile([EA, RW], F32, tag="b", name="r0")
        nc.tensor.matmul(
            r0[:], lhsT=g_h[:], rhs=WQ[:, 0:RW], start=True, stop=True
        )
        r1 = ps_b.tile([EA, RW], F32, tag="b", name="r1")
        nc.tensor.matmul(
            r1[:], lhsT=g_h[:], rhs=WQ[:, RW : 2 * RW], start=True, stop=True
        )
        tc.no_sync_barrier()

        # ---- PE transposes of my half (in R's shadow; copies on DVE) ----
        XT = sb.tile([EA, HALF, P], F16)
        pt0 = ps_a.tile([EA, HALF // 2, P], F16, tag="a", name="pt0")
        for j in range(HALF // 2):
            nc.tensor.transpose(
                out=pt0[:, j, :], in_=XA[:, j, :], identity=id_sb[:]
            )
        pt1 = ps_a.tile([EA, HALF // 2, P], F16, tag="a", name="pt1")
        for j in range(HALF // 2):
            nc.tensor.transpose(
                out=pt1[:, j, :], in_=XA[:, HALF // 2 + j, :], identity=id_sb[:]
            )
        nc.vector.tensor_copy(out=r_h[:, 0:RW], in_=r0[:])          # DVE
        nc.scalar.copy(out=r_h[:, RW : 2 * RW], in_=r1[:])          # Act
        # fence: keep the bias matmul and the XT copies from being hoisted
        # above the critical r copies
        tc.no_sync_barrier()
        nc.vector.tensor_copy(out=XT[:, 0 : HALF // 2, :], in_=pt0[:])
        nc.vector.tensor_copy(out=XT[:, HALF // 2 : HALF, :], in_=pt1[:])

        # ---- Wfin = sum_h P_h R_h + e_last bff^T (one accum group; the
        # bias matmul is LAST so a late wp cannot stall the P matmuls) ----
        wf_ps = ps_w.tile([EA, E], F32)
        for h in range(H):
            nc.tensor.matmul(
                wf_ps[:],
                lhsT=WP[:, h * EA : (h + 1) * EA],
                rhs=r_h[:, h * E : (h + 1) * E],
                start=(h == 0),
                stop=False,
            )
        nc.tensor.matmul(
            wf_ps[:],
            lhsT=WP[0:1, C_OH : C_OH + EA],
            rhs=WP[0:1, C_BF : C_BF + E],
            start=False,
            stop=True,
        )
        wf_h = sb.tile([EA, E], F16)
        nc.scalar.copy(out=wf_h[:, 0:48], in_=wf_ps[:, 0:48])
        nc.vector.tensor_copy(out=wf_h[:, 48:E], in_=wf_ps[:, 48:E])

        # ---- finals: out chunk = X_chunk @ Wfin; 2 groups of 4 chunks,
        # staged by one DVE copy and one Activation copy in parallel ----
        og0 = ps_a.tile([P, 4, E], F32, tag="a", name="og0")
        for j in range(4):
            nc.tensor.matmul(
                og0[:, j, :], lhsT=XT[:, j, :], rhs=wf_h[:],
                start=True, stop=True,
            )
        og1 = ps_a.tile([P, 4, E], F32, tag="a", name="og1")
        for j in range(4):
            nc.tensor.matmul(
                og1[:, j, :], lhsT=XT[:, 4 + j, :], rhs=wf_h[:],
                start=True, stop=True,
            )
        nc.vector.tensor_copy(out=osb[:, 0:4, :], in_=og0[:])   # DVE
        nc.scalar.copy(out=osb[:, 4:8, :], in_=og1[:])          # Act

        # ---- output store: standard HWDGE DMA on SP (its SEQ is free) ----
        nc.sync.dma_start(out=out, in_=osb2d)

    nc.compile()
    _strip_dead_const_memsets(nc)
    return nc


def _strip_dead_const_memsets(nc):
    """Drop the framework's const-AP init memsets from the entry block.

    Bass.__init__ emits four Pool-engine memsets filling const scalar tiles
    (const-float32-0.0 etc.) that nothing in this kernel reads (walrus flags
    them as reader-less). They run before the all-engine entry barrier and
    delay every queue by ~400ns.
    """
    blk = list(nc.m.functions[0].blocks)[0]
    insts = blk.instructions
    for idx in range(len(insts) - 1, -1, -1):
        i = insts[idx]
        if type(i).__name__ != "InstMemset":
            continue
        out0 = i.outs[0]
        memref = getattr(out0, "memref", "") or ""
        if memref.startswith("const-"):
            del insts[idx]


def get_nc():
    if "nc" not in _NC_CACHE:
        _NC_CACHE["nc"] = _build_nc()
    return _NC_CACHE["nc"]


def _host_weights(Wqkv, bqkv, Wff, bff):
    waug = np.concatenate(
        [np.asarray(Wqkv, np.float64), np.asarray(bqkv, np.float64)[None, :]], axis=0
    )
    Wq, Wk, Wv = waug[:, 0:E], waug[:, E : 2 * E], waug[:, 2 * E : 3 * E]
    Wff = np.asarray(Wff, np.float64)
    wqp = np.zeros((EA, WQ_COLS), np.float16)
    wpp = np.zeros((EA, WP_COLS), np.float16)
    for h in range(H):
        hd = slice(h * D, (h + 1) * D)
        Ph = Wq[:, hd] @ Wk[:, hd].T                    # [97, 97]
        Qh = SCALE * (Wv[:, hd] @ Wff[hd, :])           # [97, 96]
        wpp[0:EA, h * EA : (h + 1) * EA] = Ph.T.astype(np.float16)
        wqp[0:EA, h * E : (h + 1) * E] = Qh.astype(np.float16)
    wpp[0, C_OH + E] = 1.0                              # e_last selector row
    wpp[0, C_BF : C_BF + E] = np.asarray(bff, np.float16)
    return {"wq": wqp, "wp": wpp}


def make_in_maps(x, Wqkv, bqkv, Wff, bff):
    import ml_dtypes

    x = np.asarray(x, np.float32)
    w = _host_weights(Wqkv, bqkv, Wff, bff)
    ones = np.ones((N // 2, 1), np.float32)
    in_maps = []
    for c in range(N_CORES):
        b, h = divmod(c, 2)
        mine = np.concatenate([x[b, h * (N // 2) : (h + 1) * (N // 2)], ones], 1)
        oth = np.concatenate(
            [x[b, (1 - h) * (N // 2) : (2 - h) * (N // 2)], ones], 1
        )
        m = {
            "x": np.ascontiguousarray(mine.astype(np.float16)),
            "xb": np.ascontiguousarray(oth.astype(np.float16)),
        }
        m.update(w)
        in_maps.append(m)
    return in_maps


def assemble(results):
    out = np.empty((B, N, E), np.float32)
    for c in range(N_CORES):
        b, h = divmod(c, 2)
        out[b, h * (N // 2) : (h + 1) * (N // 2)] = np.asarray(
            results[c]["out"]
        ).reshape(N // 2, E)
    return out


def kernel(x, Wqkv, bqkv, Wff, bff):
    global LAST_RESULTS
    nc = get_nc()
    in_maps = make_in_maps(x, Wqkv, bqkv, Wff, bff)
    res = bass_utils.run_bass_kernel_spmd(
        nc, in_maps, core_ids=list(range(N_CORES))
    )
    LAST_RESULTS = res
    return assemble(res.results)


# revision 57
# speedup vs baseline: 1.0558x; 1.0558x over previous
"""Trainium2 Bass kernel for nn_MultiHeadAttention_7584912245188.

Reference computes (no softmax!):
    qkv = x @ Wqkv + bqkv ; split q,k,v ; per head: y = (q k^T / sqrt(D)) v
    out = y @ Wff + bff

Because there is no softmax, attention is linear and reassociates:
    (Q K^T) V = Q (K^T V).
With X_aug = [X | 1] ([N, 97]) and G = X_aug^T X_aug ([97, 97]), the whole
module collapses (associativity, per head h):
    out = X_aug @ Wfin,   Wfin = sum_h P_h G Q_h + e_last bff^T
    P_h = Wq_aug_h Wk_aug_h^T [97,97],  Q_h = D^-0.5 Wv_aug_h Wff_h [97,96]
P_h / Q_h are host-precomputed from the weights. On device per batch:
    G (16 accumulating matmuls over row chunks)
    R = G @ Qcat                     (3 matmuls, free dim 192 each)
    Wfin = sum_h P_h R_h + bff term  (7 PSUM-accumulating matmuls)
    out chunks = X_chunk @ Wfin      (via PE-transposed X chunks)
O(N*E^2) instead of O(N^2*D).

Sharding (8 cores): core c -> (batch b = c//2, sequence half h = c%2).
Each core receives x[b] (ones column appended host-side) rolled so "its"
half comes first, computes G from the full batch (redundantly within the
pair - cheaper than a collective), and writes only its half of the output.

This version is latency-optimized against the TimelineSim cost model
(every DMA pays descgen + 650ns DGE delay + payloads serialized on one
DMA_ENGINES device + 900ns completion-sem; every engine hop ~150ns):

- My half rides the first HWDGE slot; the other (Gram-only) half goes
  SWDGE so its descriptor generation overlaps the first HWDGE window and
  the two x payloads run back-to-back at full DMA bandwidth.
- Weights are split: the Q part (needed first, for R) rides the
  Activation engine's HWDGE slot right behind x; the P part (needed
  later, for Wfin) follows on SP. Neither gates the post-Gram chain,
  and the bias matmul is ordered LAST in the Wfin accumulation group so
  a late P-part cannot stall the P_h matmuls.
- PSUM->SBUF copies alternate between the two engines that can read
  PSUM (DVE and Activation) so the serial chain G -> R -> Wfin ->
  finals never waits on a busy copy engine; scheduler fences
  (tc.no_sync_barrier) pin the critical r0-copy ahead of the bulk XT
  transpose copies in the DVE queue.
- The framework's dead const-AP init memsets (no readers - walrus
  confirms) are stripped post-compile; they otherwise delay the entry
  barrier and every DMA by ~400ns.

A prepared-SWDGE scatter output path (trigger_dma) and fp8/DoubleRow
Gram variants were ~1.2us faster in the cost model but produced
compile-dependent numerical corruption on real hardware, so the output
uses the standard HWDGE DMA and all operands stay fp16 (f32 PSUM):
rel err 5.65e-4, deterministic across compiles.
"""

import numpy as np
from contextlib import ExitStack

import concourse.bass as bass
import concourse.tile as tile
from concourse import bacc, mybir
from concourse import bass_utils
from concourse.masks import make_identity

B, N, E = 4, 2048, 96
H = 6
D = E // H            # 16
P = 128
NCH = N // P          # 16 chunks of 128 rows
HALF = NCH // 2       # 8 chunks per core
EA = E + 1            # 97 (augmented with ones column)
SCALE = float(D) ** -0.5
F32 = mybir.dt.float32
F16 = mybir.dt.float16
I16 = mybir.dt.int16

# weight packing
WQ_COLS = H * E                      # 576  (Qcat)
C_OH = H * EA                        # 582  (PcatT | onehot | bff)
C_BF = C_OH + EA                     # 679
WP_COLS = C_BF + E                   # 775

N_CORES = 8

_NC_CACHE = {}
LAST_RESULTS = None


def _build_nc():
    nc = bacc.Bacc(
        "TRN2", target_bir_lowering=False, debug=False, num_devices=N_CORES
    )
    x = nc.dram_tensor("x", [N // 2, EA], F16, kind="ExternalInput").ap()
    xb = nc.dram_tensor("xb", [N // 2, EA], F16, kind="ExternalInput").ap()
    wq = nc.dram_tensor("wq", [EA, WQ_COLS], F16, kind="ExternalInput").ap()
    wp = nc.dram_tensor("wp", [EA, WP_COLS], F16, kind="ExternalInput").ap()
    # viewed [128, 768]: block-row p = out rows 8p..8p+7  (host reshapes)
    out = nc.dram_tensor("out", [P, HALF * E], F16, kind="ExternalOutput").ap()

    with tile.TileContext(nc) as tc, ExitStack() as ctx:
        sb = ctx.enter_context(tc.tile_pool(name="sb", bufs=1))
        ps_a = ctx.enter_context(tc.tile_pool(name="ps_a", bufs=4, space="PSUM"))
        ps_b = ctx.enter_context(tc.tile_pool(name="ps_b", bufs=3, space="PSUM"))
        ps_w = ctx.enter_context(tc.tile_pool(name="ps_w", bufs=1, space="PSUM"))

        # ---- input DMAs (per-engine emission order == queue order) ----
        # My half rides the first HWDGE slot (first payload); the other
        # half (Gram-only) goes SWDGE so its descgen overlaps; weights ride
        # the Act engine's HWDGE slot right behind them.
        xh = x.rearrange("(p j) e -> p j e", j=HALF)
        XA = sb.tile([P, HALF, EA], F16)
        nc.sync.dma_start(out=XA[:], in_=xh)                    # SP  HWDGE #1
        xbh = xb.rearrange("(p j) e -> p j e", j=HALF)
        NB1, NB2 = 5, 3
        XB1 = sb.tile([P, NB1, EA], F16)
        nc.gpsimd.dma_start(out=XB1[:], in_=xbh[:, 0:NB1, :])   # SWDGE
        # the small B tail rides the Act engine's first HWDGE slot (ready
        # in time to extend the payload train); with only 3 chunks after
        # its completion sem, the Gram tail shrinks by ~150ns
        XB2 = sb.tile([P, NB2, EA], F16)
        nc.scalar.dma_start(out=XB2[:], in_=xbh[:, NB1:HALF, :])  # Act HWDGE
        WQ = sb.tile([EA, WQ_COLS], F16)
        nc.sync.dma_start(out=WQ[:], in_=wq)                    # SP HWDGE #2
        WP = sb.tile([EA, WP_COLS], F16)
        nc.sync.dma_start(out=WP[:], in_=wp)                    # SP HWDGE #3

        # ---- Pool-engine setup work (all idle-time) ----
        id_sb = sb.tile([P, P], F16)
        make_identity(nc, id_sb[:])                             # gpsimd
        osb = sb.tile([P, HALF, E], F16)
        osb2d = osb[:].rearrange("p a b -> p (a b)")

        def Xc(c):
            if c < HALF:
                return XA[:, c, :]
            if c < HALF + NB1:
                return XB1[:, c - HALF, :]
            return XB2[:, c - HALF - NB1, :]

        # ---- G = X_aug^T X_aug: one 16-matmul fp8 PSUM accumulation ----
        g_ps = ps_b.tile([EA, EA], F32, tag="b", name="g_ps")
        for c in range(NCH):
            nc.tensor.matmul(
                g_ps[:], lhsT=Xc(c), rhs=Xc(c),
                start=(c == 0), stop=(c == NCH - 1),
            )

        # ---- g copy on Activation ----
        g_h = sb.tile([EA, EA], F16)
        nc.scalar.copy(out=g_h[:], in_=g_ps[:])

        # ---- R = G @ Qcat in 2 pieces; copies fan out to DVE/Act ----
        # (GPSIMD/Pool cannot touch PSUM, so only two copy engines exist.)
        # R runs on PE as soon as g_h lands; the transposes (whose fp16
        # input arrives later anyway) follow behind a scheduler fence so
        # they cannot be hoisted in front of R.
        r_h = sb.tile([EA, H * E], F16)
        RW = H * E // 2                                         # 288
        r0 = ps_b.tile([EA, RW], F32, tag="b", name="r0")
        nc.tensor.matmul(
            r0[:], lhsT=g_h[:], rhs=WQ[:, 0:RW], start=True, stop=True
        )
        r1 = ps_b.tile([EA, RW], F32, tag="b", name="r1")
        nc.tensor.matmul(
            r1[:], lhsT=g_h[:], rhs=WQ[:, RW : 2 * RW], start=True, stop=True
        )
        tc.no_sync_barrier()

        # ---- PE transposes of my half (in R's shadow; copies on DVE) ----
        XT = sb.tile([EA, HALF, P], F16)
        pt0 = ps_a.tile([EA, HALF // 2, P], F16, tag="a", name="pt0")
        for j in range(HALF // 2):
            nc.tensor.transpose(
                out=pt0[:, j, :], in_=XA[:, j, :], identity=id_sb[:]
            )
        pt1 = ps_a.tile([EA, HALF // 2, P], F16, tag="a", name="pt1")
        for j in range(HALF // 2):
            nc.tensor.transpose(
                out=pt1[:, j, :], in_=XA[:, HALF // 2 + j, :], identity=id_sb[:]
            )
        nc.vector.tensor_copy(out=r_h[:, 0:RW], in_=r0[:])          # DVE
        nc.scalar.copy(out=r_h[:, RW : 2 * RW], in_=r1[:])          # Act
        # fence: keep the bias matmul and the XT copies from being hoisted
        # above the critical r copies
        tc.no_sync_barrier()
        nc.vector.tensor_copy(out=XT[:, 0 : HALF // 2, :], in_=pt0[:])
        nc.vector.tensor_copy(out=XT[:, HALF // 2 : HALF, :], in_=pt1[:])

        # ---- Wfin = sum_h P_h R_h + e_last bff^T (one accum group; the
        # bias matmul is LAST so a late wp cannot stall the P matmuls) ----
        wf_ps = ps_w.tile([EA, E], F32)
        for h in range(H):
            nc.tensor.matmul(
                wf_ps[:],
                lhsT=WP[:, h * EA : (h + 1) * EA],
                rhs=r_h[:, h * E : (h + 1) * E],
                start=(h == 0),
                stop=False,
            )
        nc.tensor.matmul(
            wf_ps[:],
            lhsT=WP[0:1, C_OH : C_OH + EA],
            rhs=WP[0:1, C_BF : C_BF + E],
            start=False,
            stop=True,
        )
        wf_h = sb.tile([EA, E], F16)
        nc.scalar.copy(out=wf_h[:], in_=wf_ps[:])

        # ---- finals: out chunk = X_chunk @ Wfin; 2 groups of 4 chunks,
        # staged by one DVE copy and one Activation copy in parallel ----
        og0 = ps_a.tile([P, 4, E], F32, tag="a", name="og0")
        for j in range(4):
            nc.tensor.matmul(
                og0[:, j, :], lhsT=XT[:, j, :], rhs=wf_h[:],
                start=True, stop=True,
            )
        og1 = ps_a.tile([P, 4, E], F32, tag="a", name="og1")
        for j in range(4):
            nc.tensor.matmul(
                og1[:, j, :], lhsT=XT[:, 4 + j, :], rhs=wf_h[:],
                start=True, stop=True,
            )
        nc.vector.tensor_copy(out=osb[:, 0:4, :], in_=og0[:])   # DVE
        nc.scalar.copy(out=osb[:, 4:8, :], in_=og1[:])          # Act

        # ---- output store: standard HWDGE DMA on SP (its SEQ is free) ----
        nc.sync.dma_start(out=out, in_=osb2d)

    nc.compile()
    _strip_dead_const_memsets(nc)
    return nc


def _strip_dead_const_memsets(nc):
    """Drop the framework's const-AP init memsets from the entry block.

    Bass.__init__ emits four Pool-engine memsets filling const scalar tiles
    (const-float32-0.0 etc.) that nothing in this kernel reads (walrus flags
    them as reader-less). They run before the all-engine entry barrier and
    delay every queue by ~400ns.
    """
    blk = list(nc.m.functions[0].blocks)[0]
    insts = blk.instructions
    for idx in range(len(insts) - 1, -1, -1):
        i = insts[idx]
        if type(i).__name__ != "InstMemset":
            continue
        out0 = i.outs[0]
        memref = getattr(out0, "memref", "") or ""
        if memref.startswith("const-"):
            del insts[idx]


def get_nc():
    if "nc" not in _NC_CACHE:
        _NC_CACHE["nc"] = _build_nc()
    return _NC_CACHE["nc"]


def _host_weights(Wqkv, bqkv, Wff, bff):
    waug = np.concatenate(
        [np.asarray(Wqkv, np.float64), np.asarray(bqkv, np.float64)[None, :]], axis=0
    )
    Wq, Wk, Wv = waug[:, 0:E], waug[:, E : 2 * E], waug[:, 2 * E : 3 * E]
    Wff = np.asarray(Wff, np.float64)
    wqp = np.zeros((EA, WQ_COLS), np.float16)
    wpp = np.zeros((EA, WP_COLS), np.float16)
    for h in range(H):
        hd = slice(h * D, (h + 1) * D)
        Ph = Wq[:, hd] @ Wk[:, hd].T                    # [97, 97]
        Qh = SCALE * (Wv[:, hd] @ Wff[hd, :])           # [97, 96]
        wpp[0:EA, h * EA : (h + 1) * EA] = Ph.T.astype(np.float16)
        wqp[0:EA, h * E : (h + 1) * E] = Qh.astype(np.float16)
    wpp[0, C_OH + E] = 1.0                              # e_last selector row
    wpp[0, C_BF : C_BF + E] = np.asarray(bff, np.float16)
    return {"wq": wqp, "wp": wpp}


def make_in_maps(x, Wqkv, bqkv, Wff, bff):
    import ml_dtypes

    x = np.asarray(x, np.float32)
    w = _host_weights(Wqkv, bqkv, Wff, bff)
    ones = np.ones((N // 2, 1), np.float32)
    in_maps = []
    for c in range(N_CORES):
        b, h = divmod(c, 2)
        mine = np.concatenate([x[b, h * (N // 2) : (h + 1) * (N // 2)], ones], 1)
        oth = np.concatenate(
            [x[b, (1 - h) * (N // 2) : (2 - h) * (N // 2)], ones], 1
        )
        m = {
            "x": np.ascontiguousarray(mine.astype(np.float16)),
            "xb": np.ascontiguousarray(oth.astype(np.float16)),
        }
        m.update(w)
        in_maps.append(m)
    return in_maps


def assemble(results):
    out = np.empty((B, N, E), np.float32)
    for c in range(N_CORES):
        b, h = divmod(c, 2)
        out[b, h * (N // 2) : (h + 1) * (N // 2)] = np.asarray(
            results[c]["out"]
        ).reshape(N // 2, E)
    return out


def kernel(x, Wqkv, bqkv, Wff, bff):
    global LAST_RESULTS
    nc = get_nc()
    in_maps = make_in_maps(x, Wqkv, bqkv, Wff, bff)
    res = bass_utils.run_bass_kernel_spmd(
        nc, in_maps, core_ids=list(range(N_CORES))
    )
    LAST_RESULTS = res
    return assemble(res.results)
